# revision 26
# baseline (speedup 1.0000x reference)
"""Trainium2 Bass kernel for the merged multi-adapter LoRA layer.

Math (all fp32 reference):
    t[n,b,j,d]  = sum_m x[b,j,m] * lora_A[n,d,m]
    out[n,b,j,k] = sum_d t[n,b,j,d] * lora_B[n,k,d]

Shapes: x (4,2048,4096), lora_A (4,16,4096), lora_B (4,4096,16)
        out (4,4,2048,4096)

Sharding: data-parallel over flattened tokens (b*j = 8192 -> 1024/core on
8 cores); the tiny LoRA params are replicated. Each core reads its 2 MiB
x-shard (f16, pre-transposed on host) and writes its 32 MiB out-shard
(f16, upcast to f32 on host) -- memory-bound regime, so both streams are
stored at half precision to halve HBM traffic.

Per-core dataflow (Tile framework):
  - x arrives pre-transposed/tiled from the host as xsT[tile, p, mt, j] =
    x[tok0+j, 128*mt+p]: one fully-contiguous 1 MiB DMA per 128-token tile
    (8 KiB per partition row), so no on-chip transpose is needed at all.
  - mm1: t^T[c, tok] = sum_m A_pack[m, c] * xT[m, tok] accumulated over 32
    m-tiles; c = 32*n + d packs all 4 adapters into one matmul (columns
    16..31 of each 32-block are zero padding so mm2's lhsT/rhs partition
    bases land on 0/32/64/96).
  - mm2: out[tok, k] = sum_d t^T[32n+d, tok] * B_pack[32n+d, k]. The K=16
    contraction uses 32-row PE tile_positions; the 4 adapters' matmuls are
    issued back-to-back (kt-major) so the PE runs them concurrently in
    disjoint row-groups.
  - PSUM results are downcast-copied to f16 SBUF staging (load-balanced
    between Vector and Scalar engines) and DMA'd out as large contiguous
    stores.
"""

import numpy as np

import concourse.bacc as bacc
import concourse.bass as bass
import concourse.mybir as mybir
import concourse.tile as tile
from concourse import bass_utils
from concourse.bass import ds, ts

F32 = mybir.dt.float32
F16 = mybir.dt.float16

N_CORES = 8
B, J, M = 4, 2048, 4096
N, D, K = 4, 16, 4096
TOK = B * J                      # 8192 flattened tokens
TOK_PER_CORE = TOK // N_CORES    # 1024
TT = 128                         # token tile
N_TT = TOK_PER_CORE // TT        # 8
MT = 128                         # m (contraction) tile
N_MT = M // MT                   # 32
KT = 512                         # k tile (one PSUM bank of fp32)
OH = 2048                        # k half-width per output staging tile
ADP = 32                         # partition stride per adapter in the packed dim


def build_program():
    nc = bacc.Bacc("TRN2")

    xsT = nc.dram_tensor("xsT", [N_TT, 128, N_MT, TT], F16, kind="ExternalInput").ap()
    a_p = nc.dram_tensor("a_p", [128, N_MT, 128], F16, kind="ExternalInput").ap()
    b_p = nc.dram_tensor("b_p", [128, K], F16, kind="ExternalInput").ap()
    o = nc.dram_tensor("o", [N, TOK_PER_CORE, K], F16, kind="ExternalOutput").ap()

    with tile.TileContext(nc) as tc:
        with (
            tc.tile_pool(name="apool", bufs=1) as apool,
            tc.tile_pool(name="bpool", bufs=1) as bpool,
            tc.tile_pool(name="xpool", bufs=8) as xpool,
            tc.tile_pool(name="tpool", bufs=2) as tpool,
            tc.tile_pool(name="opool", bufs=16) as opool,
            tc.tile_pool(name="tps", bufs=1, space="PSUM") as tps_pool,
            tc.tile_pool(name="ops", bufs=7, space="PSUM") as ops_pool,
        ):
            a_sb = apool.tile([128, N_MT, 128], F16, tag="a")
            nc.scalar.dma_start(a_sb[:], a_p[:])
            b_sb = bpool.tile([128, K], F16, tag="b")
            nc.scalar.dma_start(b_sb[:], b_p[:])

            cc = [0]  # copy-engine round-robin state

            def make_group(tok_abs, half, t_sb):
                """One (128-token, 2048-k) block as 4 deferred mm2 waves.

                Each wave is 4 matmuls (one kt, all adapters back-to-back in
                distinct PE row-groups -> concurrent) + their evacuations;
                the last wave issues the stores. Waves are interleaved with
                mm1 matmuls so PSUM-bank waits never stall useful PE work.
                """
                state = {}

                def wave(kt):
                    def go():
                        if kt == 0:
                            state["osb"] = [
                                opool.tile([128, OH], F16, tag="o", name="osb")
                                for _ in range(N)
                            ]
                        osb = state["osb"]
                        for n in range(N):
                            o_ps = ops_pool.tile([128, KT], F32, tag="ops", name="ops")
                            nc.tensor.matmul(
                                o_ps[:],
                                lhsT=t_sb[ds(ADP * n, D), :],
                                rhs=b_sb[ds(ADP * n, D), ds(half * OH + kt * KT, KT)],
                                start=True,
                                stop=True,
                                tile_position=(ADP * n, 0),
                            )
                            if cc[0] % 2 == 0:
                                nc.vector.tensor_copy(osb[n][:, ts(kt, KT)], o_ps[:])
                            else:
                                nc.scalar.copy(osb[n][:, ts(kt, KT)], o_ps[:])
                            cc[0] += 1
                        if kt == OH // KT - 1:
                            for n in range(N):
                                nc.sync.dma_start(
                                    o[n, ds(tok_abs, 128), ds(half * OH, OH)],
                                    osb[n][:],
                                )
                    return go

                return [wave(kt) for kt in range(OH // KT)]

            xt = {}

            def load_x(i):
                xt[i] = xpool.tile([128, N_MT, TT], F16, tag="x", name="xt")
                nc.gpsimd.dma_start(xt[i][:], xsT[i])

            for i in range(N_TT):
                load_x(i)
            work = []
            for i in range(N_TT):
                t_ps = tps_pool.tile([128, TT], F32, tag="t", name="tps")
                for mt in range(N_MT):
                    nc.tensor.matmul(
                        t_ps[:],
                        lhsT=a_sb[:, mt, :],
                        rhs=xt[i][:, mt, :],
                        start=(mt == 0),
                        stop=(mt == N_MT - 1),
                    )
                    if mt % 4 == 3 and work:
                        work.pop(0)()
                t_sb = tpool.tile([128, TT], F16, tag="tsb", name="tsb")
                nc.vector.tensor_copy(t_sb[:], t_ps[:])
                for half in range(K // OH):
                    work.extend(make_group(i * TT, half, t_sb))
            while work:
                work.pop(0)()

    nc.compile()
    return nc


_NC_CACHE = []


def _get_nc():
    if not _NC_CACHE:
        _NC_CACHE.append(build_program())
    return _NC_CACHE[0]


def prepare_inputs(x, lora_A, lora_B):
    x = np.ascontiguousarray(np.asarray(x, dtype=np.float32)).astype(np.float16)
    lora_A = np.asarray(lora_A, dtype=np.float32)
    lora_B = np.asarray(lora_B, dtype=np.float32)

    xf = x.reshape(TOK, M)

    # a_t[m, 32n+d] = lora_A[n, d, m]; packed to [p, mt, c] so each SBUF
    # partition reads one contiguous row.
    a_t = np.zeros((M, 128), dtype=np.float32)
    for n in range(N):
        a_t[:, ADP * n : ADP * n + D] = lora_A[n].T
    a_pack = np.ascontiguousarray(
        a_t.reshape(N_MT, 128, 128).transpose(1, 0, 2)
    ).astype(np.float16)

    # b_pad[32n+d, k] = lora_B[n, k, d]
    b_pad = np.zeros((128, K), dtype=np.float16)
    for n in range(N):
        b_pad[ADP * n : ADP * n + D, :] = lora_B[n].T

    # xsT[i, p, mt, j] = x_core[i*TT + j, mt*128 + p]: per-tile transposed
    # layout so each 128-token tile is one fully contiguous 1 MiB DMA.
    in_maps = []
    for c in range(N_CORES):
        xc = xf[c * TOK_PER_CORE : (c + 1) * TOK_PER_CORE]
        xsT = np.ascontiguousarray(
            xc.reshape(N_TT, TT, N_MT, 128).transpose(0, 3, 2, 1)
        )
        in_maps.append({"xsT": xsT, "a_p": a_pack, "b_p": b_pad})
    return in_maps


def run(x, lora_A, lora_B, trace=False, **spmd_kwargs):
    nc = _get_nc()
    in_maps = prepare_inputs(x, lora_A, lora_B)
    res = bass_utils.run_bass_kernel_spmd(
        nc, in_maps, list(range(N_CORES)), trace=trace, **spmd_kwargs
    )
    o_full = np.concatenate([res.results[c]["o"] for c in range(N_CORES)], axis=1)
    return o_full.reshape(N, B, J, K).astype(np.float32), res


def kernel(x, lora_A, lora_B):
    out, _ = run(x, lora_A, lora_B)
    return out


# revision 30
# speedup vs baseline: 1.0007x; 1.0007x over previous
"""Trainium2 Bass kernel for the merged multi-adapter LoRA layer.

Math (all fp32 reference):
    t[n,b,j,d]  = sum_m x[b,j,m] * lora_A[n,d,m]
    out[n,b,j,k] = sum_d t[n,b,j,d] * lora_B[n,k,d]

Shapes: x (4,2048,4096), lora_A (4,16,4096), lora_B (4,4096,16)
        out (4,4,2048,4096)

Sharding: data-parallel over flattened tokens (b*j = 8192 -> 1024/core on
8 cores); the tiny LoRA params are replicated. Each core reads its 2 MiB
x-shard (f16, pre-transposed on host) and writes its 32 MiB out-shard
(f16, upcast to f32 on host) -- memory-bound regime, so both streams are
stored at half precision to halve HBM traffic.

Per-core dataflow (Tile framework):
  - x arrives pre-transposed/tiled from the host as xsT[tile, p, mt, j] =
    x[tok0+j, 128*mt+p]: one fully-contiguous 1 MiB DMA per 128-token tile
    (8 KiB per partition row), so no on-chip transpose is needed at all.
  - mm1: t^T[c, tok] = sum_m A_pack[m, c] * xT[m, tok] accumulated over 32
    m-tiles; c = 32*n + d packs all 4 adapters into one matmul (columns
    16..31 of each 32-block are zero padding so mm2's lhsT/rhs partition
    bases land on 0/32/64/96).
  - mm2: out[tok, k] = sum_d t^T[32n+d, tok] * B_pack[32n+d, k]. The K=16
    contraction uses 32-row PE tile_positions; the 4 adapters' matmuls are
    issued back-to-back (kt-major) so the PE runs them concurrently in
    disjoint row-groups.
  - PSUM results are downcast-copied to f16 SBUF staging (load-balanced
    between Vector and Scalar engines) and DMA'd out as large contiguous
    stores.
"""

import numpy as np

import concourse.bacc as bacc
import concourse.bass as bass
import concourse.mybir as mybir
import concourse.tile as tile
from concourse import bass_utils
from concourse.bass import ds, ts

F32 = mybir.dt.float32
F16 = mybir.dt.float16

N_CORES = 8
B, J, M = 4, 2048, 4096
N, D, K = 4, 16, 4096
TOK = B * J                      # 8192 flattened tokens
TOK_PER_CORE = TOK // N_CORES    # 1024
TT = 128                         # token tile
N_TT = TOK_PER_CORE // TT        # 8
MT = 128                         # m (contraction) tile
N_MT = M // MT                   # 32
KT = 512                         # k tile (one PSUM bank of fp32)
OH = 2048                        # k half-width per output staging tile
ADP = 32                         # partition stride per adapter in the packed dim


def build_program():
    nc = bacc.Bacc("TRN2")

    xsT = nc.dram_tensor("xsT", [N_TT, 128, N_MT, TT], F16, kind="ExternalInput").ap()
    a_p = nc.dram_tensor("a_p", [128, N_MT, 128], F16, kind="ExternalInput").ap()
    b_p = nc.dram_tensor("b_p", [128, K], F16, kind="ExternalInput").ap()
    o = nc.dram_tensor("o", [N, TOK_PER_CORE, K], F16, kind="ExternalOutput").ap()

    with tile.TileContext(nc) as tc:
        with (
            tc.tile_pool(name="apool", bufs=1) as apool,
            tc.tile_pool(name="bpool", bufs=1) as bpool,
            tc.tile_pool(name="xpool", bufs=8) as xpool,
            tc.tile_pool(name="tpool", bufs=2) as tpool,
            tc.tile_pool(name="opool", bufs=16) as opool,
            tc.tile_pool(name="tps", bufs=1, space="PSUM") as tps_pool,
            tc.tile_pool(name="ops", bufs=7, space="PSUM") as ops_pool,
        ):
            a_sb = apool.tile([128, N_MT, 128], F16, tag="a")
            nc.scalar.dma_start(a_sb[:], a_p[:])
            b_sb = bpool.tile([128, K], F16, tag="b")
            nc.scalar.dma_start(b_sb[:], b_p[:])

            cc = [0]  # copy-engine round-robin state

            def make_group(tok_abs, half, t_sb, tail=False):
                """One (128-token, 2048-k) block as 4 deferred mm2 waves.

                Each wave is 4 matmuls (one kt, all adapters back-to-back in
                distinct PE row-groups -> concurrent) + their evacuations;
                the last wave issues the stores. Waves are interleaved with
                mm1 matmuls so PSUM-bank waits never stall useful PE work.
                """
                state = {}

                def wave(kt):
                    def go():
                        if kt == 0:
                            state["osb"] = [
                                opool.tile([128, OH], F16, tag="o", name="osb")
                                for _ in range(N)
                            ]
                        osb = state["osb"]
                        for n in range(N):
                            o_ps = ops_pool.tile([128, KT], F32, tag="ops", name="ops")
                            nc.tensor.matmul(
                                o_ps[:],
                                lhsT=t_sb[ds(ADP * n, D), :],
                                rhs=b_sb[ds(ADP * n, D), ds(half * OH + kt * KT, KT)],
                                start=True,
                                stop=True,
                                tile_position=(ADP * n, 0),
                            )
                            if cc[0] % 2 == 0:
                                nc.vector.tensor_copy(osb[n][:, ts(kt, KT)], o_ps[:])
                            else:
                                nc.scalar.copy(osb[n][:, ts(kt, KT)], o_ps[:])
                            cc[0] += 1
                        if kt == OH // KT - 1:
                            # split stores so their descriptors spread over
                            # several DMA queues (a dma_start binds to one
                            # queue; the kernel tail otherwise drains through
                            # a single queue at a fraction of HBM bandwidth)
                            nsp = 4 if tail else 1
                            w = OH // nsp
                            for n in range(N):
                                for s in range(nsp):
                                    eng = nc.gpsimd if tail and s % 2 else nc.sync
                                    eng.dma_start(
                                        o[
                                            n,
                                            ds(tok_abs, 128),
                                            ds(half * OH + s * w, w),
                                        ],
                                        osb[n][:, ds(s * w, w)],
                                    )
                    return go

                return [wave(kt) for kt in range(OH // KT)]

            xt = {}

            def load_x(i):
                xt[i] = xpool.tile([128, N_MT, TT], F16, tag="x", name="xt")
                nc.gpsimd.dma_start(xt[i][:], xsT[i])

            for i in range(N_TT):
                load_x(i)
            work = []
            for i in range(N_TT):
                t_ps = tps_pool.tile([128, TT], F32, tag="t", name="tps")
                for mt in range(N_MT):
                    nc.tensor.matmul(
                        t_ps[:],
                        lhsT=a_sb[:, mt, :],
                        rhs=xt[i][:, mt, :],
                        start=(mt == 0),
                        stop=(mt == N_MT - 1),
                    )
                    if mt % 4 == 3 and work:
                        work.pop(0)()
                t_sb = tpool.tile([128, TT], F16, tag="tsb", name="tsb")
                nc.vector.tensor_copy(t_sb[:], t_ps[:])
                for half in range(K // OH):
                    work.extend(
                        make_group(i * TT, half, t_sb, tail=(i == N_TT - 1))
                    )
            while work:
                work.pop(0)()

    nc.compile()
    return nc


_NC_CACHE = []


def _get_nc():
    if not _NC_CACHE:
        _NC_CACHE.append(build_program())
    return _NC_CACHE[0]


def prepare_inputs(x, lora_A, lora_B):
    x = np.ascontiguousarray(np.asarray(x, dtype=np.float32)).astype(np.float16)
    lora_A = np.asarray(lora_A, dtype=np.float32)
    lora_B = np.asarray(lora_B, dtype=np.float32)

    xf = x.reshape(TOK, M)

    # a_t[m, 32n+d] = lora_A[n, d, m]; packed to [p, mt, c] so each SBUF
    # partition reads one contiguous row.
    a_t = np.zeros((M, 128), dtype=np.float32)
    for n in range(N):
        a_t[:, ADP * n : ADP * n + D] = lora_A[n].T
    a_pack = np.ascontiguousarray(
        a_t.reshape(N_MT, 128, 128).transpose(1, 0, 2)
    ).astype(np.float16)

    # b_pad[32n+d, k] = lora_B[n, k, d]
    b_pad = np.zeros((128, K), dtype=np.float16)
    for n in range(N):
        b_pad[ADP * n : ADP * n + D, :] = lora_B[n].T

    # xsT[i, p, mt, j] = x_core[i*TT + j, mt*128 + p]: per-tile transposed
    # layout so each 128-token tile is one fully contiguous 1 MiB DMA.
    in_maps = []
    for c in range(N_CORES):
        xc = xf[c * TOK_PER_CORE : (c + 1) * TOK_PER_CORE]
        xsT = np.ascontiguousarray(
            xc.reshape(N_TT, TT, N_MT, 128).transpose(0, 3, 2, 1)
        )
        in_maps.append({"xsT": xsT, "a_p": a_pack, "b_p": b_pad})
    return in_maps


def run(x, lora_A, lora_B, trace=False, **spmd_kwargs):
    nc = _get_nc()
    in_maps = prepare_inputs(x, lora_A, lora_B)
    res = bass_utils.run_bass_kernel_spmd(
        nc, in_maps, list(range(N_CORES)), trace=trace, **spmd_kwargs
    )
    o_full = np.concatenate([res.results[c]["o"] for c in range(N_CORES)], axis=1)
    return o_full.reshape(N, B, J, K).astype(np.float32), res


def kernel(x, lora_A, lora_B):
    out, _ = run(x, lora_A, lora_B)
    return out
